# revision 14
# baseline (speedup 1.0000x reference)
"""Multi-head attention (B=1, S=2048, H=1024, NH=16) on 8 trn2 NeuronCores.

Sharding: head-parallel. Core c owns heads {2c, 2c+1} (= 128 of the 1024
hidden dims). Each core computes its Q/K/V projection slices, the full
attention for its 2 heads, and a full-width partial of the output
projection (contraction over its 128 context dims). Host sums the 8
partials and adds the (host-folded) biases.

Masked-softmax restructure: the reference zeroes masked scores before
softmax, i.e. the numerator is m*e + (1-m) with e = exp(s/8). Using
m*e + (1-m) = m*(e-1) + 1, the kernel computes
  et = (exp(s/8) - 1) * m          (Act exp from PSUM; DVE sub at 4x,
                                    DVE mult at 2x with a bf16 mask)
and folds the "+1" into the PV matmul as a host-precomputed column-sum
of V (3 tiny rank-1 matmuls per head add colsum(V)+count to PSUM).

Engine budget per core (TimelineSim cost model):
  PE  ~73us  S (27) + PV (15) + QKVO projections (28) + transposes
  Act ~58us  exp only (reads score PSUM directly)
  DVE ~62us  (e-1)*m + normalize + tp copies + q-proj eviction
  Pool ~45us v-proj/k-proj/y evictions (idle engine in baseline)
  DMA ~70us  q,k,v 12MB + mask 8MB (bf16 for the 2x TT) + w 1MB + y 4MB
"""

import math

import numpy as np
import ml_dtypes

BF16 = ml_dtypes.bfloat16
S, H, NH, DK = 2048, 1024, 16, 64
NCORES = 8
HPC = NH // NCORES          # heads per core = 2
DPC = HPC * DK              # head dims per core = 128
KC = H // 128               # contraction chunks = 8
TP = S // 512               # 512-wide token panels = 4
JC = S // 128               # 128-wide key chunks = 16
VA = DK + 1                 # v columns + ones column = 65

_CACHE = {}


def _oslc(ic):
    """o_ps column offset for ic-th 65-wide slice: 7 slices per 512-fp32
    PSUM bank so no matmul crosses a bank boundary."""
    b, r = divmod(ic, 7)
    return b * 512 + r * VA


def _build_program():
    """Build + compile the (identical) per-core Bass program."""
    from contextlib import ExitStack

    import concourse.bacc as bacc
    import concourse.tile as tile
    from concourse import mybir

    dt = mybir.dt
    AF = mybir.ActivationFunctionType

    nc = bacc.Bacc("TRN2", target_bir_lowering=False, debug=False)

    # token-quarter-major x layouts: [4q][128 p][8 c][512 i] flattened
    qx_d = nc.dram_tensor("qx", [4 * 128, KC * 512], dt.bfloat16, kind="ExternalInput").ap()
    kx_d = nc.dram_tensor("kx", [4 * 128, KC * 512], dt.bfloat16, kind="ExternalInput").ap()
    vx_d = nc.dram_tensor("vx", [4 * 128, KC * 512], dt.bfloat16, kind="ExternalInput").ap()
    maskT_d = nc.dram_tensor("maskT", [S, S], dt.bfloat16, kind="ExternalInput").ap()
    wq_d = nc.dram_tensor("wq", [128, KC * DPC], dt.bfloat16, kind="ExternalInput").ap()
    # wk | wv | wo | ident packed: one DMA for the non-critical weights
    wpk_d = nc.dram_tensor("wpk", [128, 3 * H + 128], dt.bfloat16, kind="ExternalInput").ap()
    bqk_d = nc.dram_tensor("bqk", [DPC, 2], dt.float32, kind="ExternalInput").ap()
    vcr_d = nc.dram_tensor("vcr", [1, HPC * 7 * VA], dt.bfloat16, kind="ExternalInput").ap()
    yT_d = nc.dram_tensor("yT", [H, S], dt.bfloat16, kind="ExternalOutput").ap()

    with tile.TileContext(nc) as tc, ExitStack() as ctx:
        cp = ctx.enter_context(tc.tile_pool(name="const", bufs=1))
        e_p = ctx.enter_context(tc.tile_pool(name="ex", bufs=3))
        ot_p = ctx.enter_context(tc.tile_pool(name="otok", bufs=2))
        rc_p = ctx.enter_context(tc.tile_pool(name="recip", bufs=3))

        # ---- DMA priority: wq, qx quarters (PE-critical), rest behind ----
        wq_sb = cp.tile([128, KC * DPC], dt.bfloat16, tag="wq")
        nc.sync.dma_start(out=wq_sb, in_=wq_d)
        ones_col = cp.tile([1, 128], dt.bfloat16, tag="ones")
        nc.vector.memset(ones_col, 1.0)
        warm = cp.tile([128, 512], dt.bfloat16, tag="warm")
        nc.vector.memset(warm, 0.0)

        qT_sb = cp.tile([128, S], dt.bfloat16, tag="qTs")
        kT_sb = cp.tile([128, S], dt.bfloat16, tag="kTs")
        vaug = cp.tile([128, JC * (HPC * VA)], dt.bfloat16, tag="vaug")
        oT_sb = [cp.tile([128, 512], dt.bfloat16, tag=f"oTp{p}", name=f"oTp{p}")
                 for p in range(TP)]
        # y pair-tiles: cols 0:1024 = even nn, 1024:2048 = odd nn (one panel-half)
        y_sb = [cp.tile([128, 2048], dt.bfloat16, tag=f"ysb{pr}", name=f"ysb{pr}")
                for pr in range(4)]

        import concourse.bass as bass_mod

        with tc.tile_pool(name="qxin", bufs=1) as qx_pool, \
             tc.tile_pool(name="ps_proj", bufs=1, space="PSUM") as pq:
            qin = [None] * TP
            for qq in range(2):
                t_ = qx_pool.tile([128, KC * 512], dt.bfloat16, tag=f"xq{qq}",
                                  name=f"xq{qq}")
                nc.sync.dma_start(out=t_, in_=qx_d[qq * 128:(qq + 1) * 128, :])
                qin[qq] = t_
            wpk = cp.tile([128, 3 * H + 128], dt.bfloat16, tag="wpk")
            nc.sync.dma_start(out=wpk, in_=wpk_d)
            w_sb = {"wq": wq_sb, "wk": wpk[:, 0:H], "wv": wpk[:, H:2 * H]}
            wo_sb = wpk[:, 2 * H:3 * H]
            ident = wpk[:, 3 * H:3 * H + 128]
            for qq in range(2, TP):
                t_ = qx_pool.tile([128, KC * 512], dt.bfloat16, tag=f"xq{qq}",
                                  name=f"xq{qq}")
                nc.sync.dma_start(out=t_, in_=qx_d[qq * 128:(qq + 1) * 128, :])
                qin[qq] = t_
            bqk_sb = cp.tile([DPC, 2], dt.float32, tag="bqk")
            nc.sync.dma_start(out=bqk_sb, in_=bqk_d)
            bq_sb = bqk_sb[:, 0:1]
            bk_sb = bqk_sb[:, 1:2]
            vcr_sb = cp.tile([1, HPC * 7 * VA], dt.bfloat16, tag="vcr")
            nc.sync.dma_start(out=vcr_sb, in_=vcr_d)
            # preload the Exp table + warm up the PE p-state ramp while the
            # first q quarter streams in
            etab = cp.tile([1, 2], dt.bfloat16, tag="etab")
            nc.scalar.activation(etab, ones_col[:, 0:2], AF.Exp)
            wps = pq.tile([128, 512], dt.float32, tag="warmp", name="warmp")
            for i in range(10):
                nc.tensor.matmul(
                    wps, lhsT=warm[:, 0:128], rhs=warm,
                    start=(i == 0), stop=(i == 9),
                )
            # vaug ones-columns: one strided memset (col 64 of every 65-slice)
            vaug_ones = bass_mod.AP(
                tensor=vaug.tensor,
                offset=vaug.offset + DK,
                ap=[vaug.ap[0], [VA, JC * HPC]],
            )
            nc.vector.memset(vaug_ones, 1.0)

            for p in range(TP):
                ps = pq.tile([128, 512], dt.float32, tag=f"pq{p}", name=f"pq{p}")
                for kk in range(KC):
                    nc.tensor.matmul(
                        ps,
                        lhsT=w_sb["wq"][:, kk * DPC:(kk + 1) * DPC],
                        rhs=qin[p][:, kk * 512:(kk + 1) * 512],
                        start=(kk == 0),
                        stop=(kk == KC - 1),
                    )
                nc.vector.tensor_scalar_add(
                    qT_sb[:, p * 512:(p + 1) * 512], ps, bq_sb
                )

        # ---- k/v/mask tiles + DMAs, deadline-ordered for the h0 j-loop.
        # k/v get fresh SBUF; late mask blocks (8-15) reuse the freed q
        # space (their DMAs wait on the last q-proj read, which is fine
        # since they are consumed late in the h0 loop). ----
        mask_lo = cp.tile([128, 8 * S], dt.bfloat16, tag="mask_lo")
        kv_pool = ctx.enter_context(tc.tile_pool(name="kvin", bufs=1))
        mask_hi = kv_pool.tile([128, 8 * S], dt.bfloat16, tag="mask_hi")
        kin = [None] * TP
        vin = [None] * TP

        def mask_ap(j):
            t_ = mask_lo if j < 8 else mask_hi
            return t_[:, (j % 8) * S:(j % 8 + 1) * S]

        def x_quarter(which, qq):
            d, store, pre = {
                "k": (kx_d, kin, "k"), "v": (vx_d, vin, "v"),
            }[which]
            t_ = kv_pool.tile([128, KC * 512], dt.bfloat16, tag=f"x{pre}{qq}",
                              name=f"x{pre}{qq}")
            nc.sync.dma_start(out=t_, in_=d[qq * 128:(qq + 1) * 128, :])
            store[qq] = t_

        def mask_pair(j):
            """DMA mask blocks j, j+1 in one transfer."""
            t_ = mask_lo if j < 8 else mask_hi
            nc.sync.dma_start(
                out=t_[:, (j % 8) * S:(j % 8 + 2) * S].rearrange(
                    "p (a i) -> p a i", a=2
                ),
                in_=maskT_d[j * 128:(j + 2) * 128, :].rearrange(
                    "(a p) i -> p a i", p=128
                ),
            )

        for tok in ("k0 m0 v0 m2 k1 m4 v1 m6 k2 m8 v2 m10 k3 m12 v3 "
                    "m14").split():
            if tok[0] == "m":
                mask_pair(int(tok[1:]))
            else:
                x_quarter(tok[0], int(tok[1:]))

        # ---- attention; V projection and just-in-time K-panel projections
        # ride the h=0 j-loop. PSUM banks: s 2x2 + o 3 + misc 1 = 8
        if True:
            with tc.tile_pool(name="ps_misc", bufs=1, space="PSUM") as pm, \
                 tc.tile_pool(name="ps_s", bufs=2, space="PSUM") as ps_p, \
                 tc.tile_pool(name="ps_o", bufs=1, space="PSUM") as po_p:

                def k_proj_panel(p):
                    ps = pm.tile([128, 512], dt.float32, tag="misc", name=f"pk{p}")
                    for kk in range(KC):
                        nc.tensor.matmul(
                            ps[:, 0:512],
                            lhsT=w_sb["wk"][:, kk * DPC:(kk + 1) * DPC],
                            rhs=kin[p][:, kk * 512:(kk + 1) * 512],
                            start=(kk == 0),
                            stop=(kk == KC - 1),
                        )
                    nc.vector.tensor_scalar_add(
                        kT_sb[:, p * 512:(p + 1) * 512], ps[:, 0:512], bk_sb
                    )

                def v_proj_chunk(t):
                    """Token-chunk t of the V projection into vaug (Act evicts)."""
                    ps = pm.tile([128, 512], dt.float32, tag="misc", name=f"pv{t}")
                    qq, ts_ = divmod(t, 4)
                    for kk in range(KC):
                        nc.tensor.matmul(
                            ps[:, 0:DPC],
                            lhsT=vin[qq][:, kk * 512 + ts_ * 128: kk * 512 + (ts_ + 1) * 128],
                            rhs=w_sb["wv"][:, kk * DPC:(kk + 1) * DPC],
                            start=(kk == 0),
                            stop=(kk == KC - 1),
                        )
                    base = t * (HPC * VA)
                    for h in range(HPC):
                        nc.scalar.copy(
                            vaug[:, base + h * VA: base + h * VA + DK],
                            ps[:, h * DK:(h + 1) * DK],
                        )

                def pv_mms(h, j, et, o_ps):
                    for ic in range(JC):
                        nc.tensor.matmul(
                            o_ps[:, _oslc(ic): _oslc(ic) + VA],
                            lhsT=et[:, ic * 128:(ic + 1) * 128],
                            rhs=vaug[:, j * (HPC * VA) + h * VA: j * (HPC * VA) + (h + 1) * VA],
                            start=(j == 0 and ic % 7 == 0),
                            stop=False,
                        )

                k_proj_panel(0)
                for h in range(HPC):
                    hs = h * DK
                    o_ps = po_p.tile([128, 1536], dt.float32, tag="ops")
                    pend = None  # (j, et) whose PV matmuls are not yet emitted
                    for j in range(JC):
                        et = e_p.tile([128, S], dt.bfloat16, tag="et")
                        for half in range(2):
                            s_ps = ps_p.tile([128, 1024], dt.float32, tag="sps")
                            for q in range(2):
                                pi = half * 2 + q
                                nc.tensor.matmul(
                                    s_ps[:, q * 512:(q + 1) * 512],
                                    lhsT=kT_sb[hs:hs + DK, j * 128:(j + 1) * 128],
                                    rhs=qT_sb[hs:hs + DK, pi * 512:(pi + 1) * 512],
                                    start=True,
                                    stop=True,
                                )
                            eh = et[:, half * 1024:(half + 1) * 1024]
                            nc.scalar.activation(eh, s_ps, AF.Exp, scale=1.0 / math.sqrt(DK))
                            nc.vector.tensor_scalar_sub(eh, eh, 1.0)
                        nc.vector.tensor_mul(et, et, mask_ap(j))
                        # software pipeline: PE emits S(j+1) before PV(j), so
                        # the S->exp->mask->PV chain doesn't serialize per j.
                        # V-projection chunks and just-in-time K-panel
                        # projections ride the same pipeline slot.
                        if h == 0 and j in (1, 5, 9):
                            k_proj_panel(j // 4 + 1)
                        if pend is not None:
                            if h == 0:
                                v_proj_chunk(pend[0])
                            pv_mms(h, pend[0], pend[1], o_ps)
                        pend = (j, et)
                    if h == 0:
                        v_proj_chunk(pend[0])
                    pv_mms(h, pend[0], pend[1], o_ps)
                    # +1 correction: colsum(V)+count via rank-1 matmuls,
                    # one per 65-wide slice (closes each PSUM group)
                    for ic in range(JC):
                        nc.tensor.matmul(
                            o_ps[:, _oslc(ic): _oslc(ic) + VA],
                            lhsT=ones_col,
                            rhs=vcr_sb[:, h * 7 * VA + (ic % 7) * VA:
                                       h * 7 * VA + (ic % 7) * VA + VA],
                            start=False,
                            stop=(ic in (6, 13, 15)),
                        )
                    # epilogue: per PSUM bank (7 ic-slices), batch-reciprocal
                    # the denominator columns and batch-normalize via a 3D
                    # strided AP with the recip broadcast (step-0) over DK
                    ot_big = ot_p.tile([128, JC * DK], dt.bfloat16, tag="ot")
                    for b in range(3):
                        n_ic = (7, 7, 2)[b]
                        rc = rc_p.tile([128, 8], dt.float32, tag="rc", name=f"rc{h}_{b}")
                        den = bass_mod.AP(
                            tensor=o_ps.tensor,
                            offset=o_ps.offset + b * 512 + DK,
                            ap=[o_ps.ap[0], [VA, n_ic]],
                        )
                        nc.vector.reciprocal(rc[:, :n_ic], den)
                        src_ap = bass_mod.AP(
                            tensor=o_ps.tensor,
                            offset=o_ps.offset + b * 512,
                            ap=[o_ps.ap[0], [VA, n_ic], [1, DK]],
                        )
                        rcb = bass_mod.AP(
                            tensor=rc.tensor,
                            offset=rc.offset,
                            ap=[rc.ap[0], [1, n_ic], [0, DK]],
                        )
                        dst = ot_big[:, b * 7 * DK:(b * 7 + n_ic) * DK].rearrange(
                            "p (a d) -> p a d", d=DK
                        )
                        nc.vector.tensor_mul(dst, src_ap, rcb)
                    for ic in range(JC):
                        ot = ot_big[:, ic * DK:(ic + 1) * DK]
                        if h == HPC - 1 and ic % 2 == 0:
                            tp = ps_p.tile([DK, 128], dt.bfloat16, tag="sps", name=f"tp{h}_{ic}")
                        else:
                            tp = pm.tile([DK, 128], dt.bfloat16, tag="misc", name=f"tp{h}_{ic}")
                        nc.tensor.transpose(tp, ot, ident)
                        nc.vector.tensor_copy(
                            oT_sb[ic // 4][hs:hs + DK, (ic % 4) * 128:(ic % 4 + 1) * 128],
                            tp,
                        )
                        if h == HPC - 1 and ic % 4 == 3:
                            p = ic // 4
                            # O-projection: 4 pair-matmuls into [128,1024]
                            # PSUM, one wide strided eviction per pair
                            # (cols 0:512 -> even nn, 512:1024 -> odd nn)
                            for pr in range(4):
                                y_ps = ps_p.tile(
                                    [128, 1024], dt.float32, tag="sps",
                                    name=f"y{p}_{pr}",
                                )
                                for e in range(2):
                                    nc.tensor.matmul(
                                        y_ps[:, e * 512:(e + 1) * 512],
                                        lhsT=wo_sb[:, (2 * pr + e) * 128:
                                                   (2 * pr + e + 1) * 128],
                                        rhs=oT_sb[p],
                                        start=True,
                                        stop=True,
                                    )
                                ydst = bass_mod.AP(
                                    tensor=y_sb[pr].tensor,
                                    offset=y_sb[pr].offset + (p % 2) * 512,
                                    ap=[y_sb[pr].ap[0], [1024, 2], [1, 512]],
                                )
                                if pr % 2 == 0:
                                    nc.scalar.copy(ydst, y_ps.rearrange(
                                        "p (a i) -> p a i", a=2))
                                else:
                                    nc.vector.tensor_copy(ydst, y_ps.rearrange(
                                        "p (a i) -> p a i", a=2))
                                if p % 2 == 1:
                                    nc.sync.dma_start(
                                        out=yT_d[2 * pr * 128:(2 * pr + 2) * 128,
                                                 (p // 2) * 1024:(p // 2 + 1) * 1024
                                                 ].rearrange("(a p) i -> p a i", p=128),
                                        in_=y_sb[pr].rearrange("p (a i) -> p a i", a=2),
                                    )

    nc.compile()
    return nc


def get_program():
    if "nc" not in _CACHE:
        _CACHE["nc"] = _build_program()
    return _CACHE["nc"]


def _wshuf(wT):
    """[1024 k, 128 n] -> [128 p, KC*128] with chunk kk at cols kk*128."""
    return np.ascontiguousarray(
        wT.reshape(KC, 128, DPC).transpose(1, 0, 2).reshape(128, KC * DPC)
    ).astype(BF16)


def _xquarters(x):
    """[S tok, H feat] fp32 -> [4*128, 8*512] bf16 token-quarter-major:
    [q][p][c][i] with element = x[512q + i, 128c + p]."""
    xT = np.asarray(x, np.float32).T            # [H, S]
    x4 = xT.reshape(KC, 128, TP, 512)           # [c, p, q, i]
    return np.ascontiguousarray(
        x4.transpose(2, 1, 0, 3).reshape(TP * 128, KC * 512)
    ).astype(BF16)


def make_in_maps(query, key, value, attention_mask, Wq, bq, Wk, bk, Wv, Wo):
    """Host-side sharding: per-core input dicts."""
    qx = _xquarters(np.asarray(query, np.float32)[0])
    kx = _xquarters(np.asarray(key, np.float32)[0])
    vx = _xquarters(np.asarray(value, np.float32)[0])
    maskT = np.ascontiguousarray(
        np.asarray(attention_mask, np.float32)[0, 0].T
    ).astype(BF16)
    # colsum(V)+count per core-head, device-matched: colsum over bf16(V)
    v_f = np.asarray(value, np.float32)[0]
    Wv_f = np.asarray(Wv, np.float32)

    in_maps = []
    for c in range(NCORES):
        ns = slice(c * DPC, (c + 1) * DPC)
        vproj = (v_f @ Wv_f[ns].T).astype(BF16).astype(np.float32)  # [S, 128]
        vcol = vproj.sum(axis=0)                                    # [128]
        vcr = np.zeros((1, HPC * 7 * VA), np.float32)
        for h in range(HPC):
            tile65 = np.concatenate([vcol[h * DK:(h + 1) * DK], [float(S)]])
            vcr[0, h * 7 * VA:(h + 1) * 7 * VA] = np.tile(tile65, 7)
        wpk = np.concatenate(
            [
                _wshuf(np.asarray(Wk, np.float32)[ns].T),
                _wshuf(Wv_f[ns].T),
                np.ascontiguousarray(np.asarray(Wo, np.float32)[:, ns].T).astype(BF16),
                np.eye(128, dtype=BF16),
            ],
            axis=1,
        )
        bqk = np.stack(
            [np.asarray(bq, np.float32)[ns], np.asarray(bk, np.float32)[ns]],
            axis=1,
        )
        in_maps.append(
            {
                "qx": qx,
                "kx": kx,
                "vx": vx,
                "maskT": maskT,
                "wq": _wshuf(np.asarray(Wq, np.float32)[ns].T),
                "wpk": np.ascontiguousarray(wpk),
                "bqk": np.ascontiguousarray(bqk),
                "vcr": vcr.astype(BF16),
            }
        )
    return in_maps


def combine_outputs(results, Wv_bias, Wo, bo):
    """Sum per-core partial yT's (bf16 -> fp32), add host-folded biases."""
    acc = np.zeros((H, S), np.float32)
    for r in results:
        acc += r["yT"].astype(np.float32)
    bias = np.asarray(bo, np.float32) + np.asarray(Wv_bias, np.float32) @ np.asarray(
        Wo, np.float32
    ).T
    return (acc.T + bias[None, :]).astype(np.float32)[None]


def kernel(
    query,
    key,
    value,
    attention_mask,
    Wq,
    bq,
    Wk,
    bk,
    Wv,
    bv,
    Wo,
    bo,
    head,
    hidden_size,
):
    from concourse.bass_utils import run_bass_kernel_spmd

    nc = get_program()
    in_maps = make_in_maps(
        query, key, value, attention_mask, Wq, bq, Wk, bk, Wv, Wo
    )
    res = run_bass_kernel_spmd(nc, in_maps, list(range(NCORES)))
    return combine_outputs(res.results, bv, Wo, bo)
